# revision 23
# baseline (speedup 1.0000x reference)
"""NT-Xent loss on 8 Trainium2 NeuronCores (Bass/Tile), fp8 + symmetric.

Reference computation (B=4096, D=1024, T=0.5):
    x  = concat(z_i, z_j); xn = x / ||x||; sim = xn @ xn.T
    logits = sim / T, diag masked to -inf
    loss = -mean(log_softmax(logits)[i, target(i)]), target(i) = i ^ 1

Sharding + symmetry: core c owns rows [1024c, 1024(c+1)). exp(sim/T) is
symmetric, so each core computes only rotated column-blocks r = 0..3
fully plus the upper sub-block triangle of r = 4 (sub-blocks (i,j),
j >= i, of the 8x8 128-col grid). The mirrored contributions are
recovered from per-column sums of the computed exp tiles:
  - blocks r = 1..3: full column sums -> rows of core c+r
  - block r = 4: column sums EXCLUDING the diagonal sub-blocks (those
    pairs are computed by both endpoints' own sweeps) -> rows of c+4
The host adds each core's row-sum partials and the received column-sum
partials, subtracts ediag, and finishes loss = mean(log(den) -
log(etarg)). This is the final cross-core reduction the sharding hint
assigns to an all-reduce; it is O(N) scalar work.

Per-core pipeline: STAGE (DMA fp8 chunk, squares split ACT/DVE/GpSimd,
DoubleRow-ones partition-sum -> s), NEWTON (batched constant-seed
rsqrt on DVE, 4 iterations; s ~ chi^2(1024) so seed 1/32 converges),
NORM (K=1 matmul broadcasts 16/||x||, DVE/GpSimd multiply -> fp8),
SWEEP (8 DoubleRow matmuls per m-tile into a 2-bank [128,1024] PSUM
tile, one ACT exp with per-partition scale inv/8 + f32 row-sum accum;
colsum matmuls lag one m-tile so the in-order PE stream never waits on
ACT). Diag/target extracted from the t=0 pair by mask multiply+reduce.

fp8 rationale: rel-err budget is 2e-2; e4m3 quantization perturbs sim
by ~2e-3 which lands ~2e-5 on the loss. DoubleRow (2 contraction tiles
per pass) is ~1.5x over bf16 at FD=512, and fp8 quarters input DMA.
"""

import numpy as np
import ml_dtypes
from contextlib import ExitStack

import concourse.bass as bass
import concourse.tile as tile
from concourse import bacc, mybir
from concourse.bass_utils import run_bass_kernel_spmd

F32 = mybir.dt.float32
BF16 = mybir.dt.bfloat16
F8 = mybir.dt.float8e4

B = 4096
D = 1024
N = 2 * B            # 8192 rows total
NCORES = 8
RPC = N // NCORES    # 1024 rows per core
KT = D // 128        # 8 contraction partition-tiles
KP = KT // 2         # 4 DoubleRow contraction pairs
MT = RPC // 128      # 8 row tiles per core
CHUNK = 512
NCH = 10             # computed column chunks: blocks r=0..4
NPAIR = 5            # chunk pairs (sweeps)
CSB = 8              # colsum chunks (blocks r=1..4 -> chunks 2..9)

_NC_CACHE = {}
LAST_RESULTS = None  # BassKernelResults of the most recent run (for test.py)


def _build_program():
    nc = bacc.Bacc("TRN2", target_bir_lowering=False, debug=False)

    x8 = nc.dram_tensor("x8", [D, NCH * CHUNK], F8, kind="ExternalInput")
    msk = nc.dram_tensor("msk", [128, 256], F32, kind="ExternalInput")
    rsum_o = nc.dram_tensor("rsum", [128, MT], F32, kind="ExternalOutput")
    ediag_o = nc.dram_tensor("ediag", [128, MT], F32, kind="ExternalOutput")
    etarg_o = nc.dram_tensor("etarg", [128, MT], F32, kind="ExternalOutput")
    csum_o = nc.dram_tensor("csum", [1, CSB * CHUNK], F32, kind="ExternalOutput")

    ADD = mybir.AluOpType.add
    MULT = mybir.AluOpType.mult
    EXP = mybir.ActivationFunctionType.Exp
    SQ = mybir.ActivationFunctionType.Square
    DR = mybir.MatmulPerfMode.DoubleRow

    with tile.TileContext(nc) as tc, ExitStack() as ctx:
        consts = ctx.enter_context(tc.tile_pool(name="consts", bufs=1))
        own_pool = ctx.enter_context(tc.tile_pool(name="own", bufs=1))
        raw_pool = ctx.enter_context(tc.tile_pool(name="raw", bufs=8))
        sq_pool = ctx.enter_context(tc.tile_pool(name="sq", bufs=8))
        xnc_pool = ctx.enter_context(tc.tile_pool(name="xnc", bufs=6))
        sv_pool = ctx.enter_context(tc.tile_pool(name="sv", bufs=4))
        inv_pool = ctx.enter_context(tc.tile_pool(name="invb", bufs=3))
        exp_pool = ctx.enter_context(tc.tile_pool(name="exp", bufs=4))
        scr_pool = ctx.enter_context(tc.tile_pool(name="scr", bufs=2))
        nt_pool = ctx.enter_context(tc.tile_pool(name="nt", bufs=2))
        stat_pool = ctx.enter_context(tc.tile_pool(name="stat", bufs=1))
        dram_pool = ctx.enter_context(tc.tile_pool(name="dram", bufs=1, space="DRAM"))
        small_pool = ctx.enter_context(tc.tile_pool(name="small", bufs=4))
        ps_s = ctx.enter_context(tc.tile_pool(name="ps_s", bufs=1, space="PSUM"))
        ps_b = ctx.enter_context(tc.tile_pool(name="ps_b", bufs=1, space="PSUM"))
        ps_cs = ctx.enter_context(tc.tile_pool(name="ps_cs", bufs=1, space="PSUM"))
        ps_g = ctx.enter_context(tc.tile_pool(name="ps_g", bufs=2, space="PSUM"))

        msk_sb = consts.tile([128, 256], F32)
        nc.sync.dma_start(msk_sb[:], msk[:])
        ones_k1 = consts.tile([1, 128], BF16)
        nc.vector.memset(ones_k1[:], 1.0)
        ones_m1 = consts.tile([128, 1], BF16)
        nc.vector.memset(ones_m1[:], 1.0)
        # DoubleRow ones weights: k-pair step must be 16 B aligned.
        ones_dr = consts.tile([128, 2, 16], F8)
        nc.vector.memset(ones_dr[:], 1.0)

        x8_own = own_pool.tile([128, KT, RPC], F8)

        inv2_rm = stat_pool.tile([128, MT], F32)
        s_dram = dram_pool.tile([1, NCH * CHUNK], F32)
        inv_dram = dram_pool.tile([1, NCH * CHUNK], BF16)

        esum = stat_pool.tile([128, MT, NPAIR], F32)
        ediag = stat_pool.tile([128, MT], F32)
        etarg = stat_pool.tile([128, MT], F32)
        csum_sb = stat_pool.tile([1, CSB * CHUNK], F32)
        # cols [3072, 3200) (head of the r=4 block, no strict-upper source)
        # are never written by the colsum drains; zero everything once.
        nc.vector.memset(csum_sb[:], 0.0)

        x8_r = x8[:].rearrange("(k p) n -> p k n", k=KT)

        def stage_chunk(j):
            """DMA raw fp8 chunk j, square it (DVE/GpSimd/ACT split),
            DoubleRow-ones partition-sum -> s_dram."""
            csl = slice(CHUNK * j, CHUNK * (j + 1))
            if j < 2:
                raw = x8_own[:, :, csl]
            else:
                raw_t = raw_pool.tile([128, KT, CHUNK], F8)
                raw = raw_t[:]
            half = KT // 2
            nc.sync.dma_start(raw[:, 0:half, :], x8_r[:, 0:half, csl])
            nc.sync.dma_start(raw[:, half:KT, :], x8_r[:, half:KT, csl])
            sq = sq_pool.tile([128, KT, CHUNK], F8)
            # engine split; prologue chunks lean on ACT/DVE so the slow
            # GpSimd leg doesn't delay the first Newton batches.
            na, nd = (4, 3) if j < 4 else (3, 2)
            nc.scalar.activation(sq[:, 0:na, :], raw[:, 0:na, :], SQ)
            for k in range(na, na + nd):
                nc.vector.tensor_mul(sq[:, k, :], raw[:, k, :], raw[:, k, :])
            for k in range(na + nd, KT):
                nc.gpsimd.tensor_mul(sq[:, k, :], raw[:, k, :], raw[:, k, :])
            s_ps = ps_s.tile([1, CHUNK], F32)
            for kk in range(KP):
                nc.tensor.matmul(
                    s_ps[:], lhsT=ones_dr[:, :, 0:1], rhs=sq[:, 2 * kk:2 * kk + 2, :],
                    start=(kk == 0), stop=(kk == KP - 1), perf_mode=DR,
                )
            s_sb = sv_pool.tile([1, CHUNK], F32)
            nc.scalar.copy(s_sb[:], s_ps[:])
            nc.gpsimd.dma_start(s_dram[0:1, csl], s_sb[:])
            return raw

        def newton_batch(c0, nch):
            """inv = rsqrt(s) for nch chunks starting at chunk c0, on DVE."""
            base = CHUNK * c0
            bw = nch * CHUNK // 128
            da = s_dram[:]
            s_bat = nt_pool.tile([128, bw], F32)
            nc.gpsimd.dma_start(
                s_bat[:],
                bass.AP(tensor=da.tensor, offset=da.offset + base,
                        ap=[[1, 128], [128, bw]]))
            y = nt_pool.tile([128, bw], F32)
            nc.vector.memset(y[:], 1.0 / 32.0)
            t = nt_pool.tile([128, bw], F32)
            for _ in range(4):
                nc.vector.tensor_mul(t[:], y[:], y[:])
                nc.vector.tensor_mul(t[:], t[:], s_bat[:])
                nc.vector.tensor_scalar(
                    out=t[:], in0=t[:], scalar1=-0.5, scalar2=1.5,
                    op0=MULT, op1=ADD)
                nc.vector.tensor_mul(y[:], y[:], t[:])
            if c0 == 0:
                nc.vector.tensor_scalar_mul(inv2_rm[:], y[:, 0:MT], 0.125)
            y16 = nt_pool.tile([128, bw], BF16)
            nc.vector.tensor_scalar_mul(y16[:], y[:], 16.0)
            di = inv_dram[:]
            nc.gpsimd.dma_start(
                bass.AP(tensor=di.tensor, offset=di.offset + base,
                        ap=[[1, 128], [128, bw]]),
                y16[:])

        def norm_chunk(j, raw):
            csl = slice(CHUNK * j, CHUNK * (j + 1))
            inv_sl = sv_pool.tile([1, CHUNK], BF16)
            nc.sync.dma_start(inv_sl[:], inv_dram[0:1, csl])
            b_ps = ps_b.tile([128, CHUNK], F32)
            nc.tensor.matmul(b_ps[:], lhsT=ones_k1[:], rhs=inv_sl[:],
                             start=True, stop=True)
            invn = inv_pool.tile([128, CHUNK], BF16)
            nc.scalar.copy(invn[:], b_ps[:])
            xnc = xnc_pool.tile([128, KT, CHUNK], F8)
            nd = 4 if j % 2 == 0 else 5
            for k in range(nd):
                nc.vector.tensor_mul(xnc[:, k, :], raw[:, k, :], invn[:])
            for k in range(nd, KT):
                nc.gpsimd.tensor_mul(xnc[:, k, :], raw[:, k, :], invn[:])
            return xnc

        def sweep(t, xnc_a, xnc_b):
            """m-tiles against chunk pair (2t, 2t+1). t=4 is the block-4
            triangle: m-tile m covers block-local cols [128m, 1024).
            Colsums (pairs t>=1) accumulate over m in PSUM; t=4 colsums
            exclude the diagonal sub-block of each m. The colsum matmul
            for m is emitted after sim(m+1) so the in-order PE stream
            never waits on ACT's exp(m)."""
            tri = (t == NPAIR - 1)
            if t >= 1:
                cs_a = ps_cs.tile([1, CHUNK], F32)
                cs_b = ps_cs.tile([1, CHUNK], F32)

            def emit_cs(m, esb):
                # column sums for the mirrored rows. For the triangle
                # pair, skip the diagonal sub-block: start at 128(m+1).
                cs_off = 128 * (m + 1) if tri else 0
                for half, cs in ((0, cs_a), (1, cs_b)):
                    lo = max(cs_off - half * CHUNK, 0)
                    if lo >= CHUNK:
                        continue
                    first_m = 0
                    last_m = (2 if half == 0 else 6) if tri else MT - 1
                    if m > last_m:
                        continue
                    nc.tensor.matmul(
                        cs[0:1, lo:CHUNK], lhsT=ones_m1[:],
                        rhs=esb[:, half * CHUNK + lo:(half + 1) * CHUNK],
                        start=(m == first_m), stop=(m == last_m),
                        skip_group_check=True,
                    )

            prev = None
            for m in range(MT):
                off = 128 * m if tri else 0   # block-local start col
                g = ps_g.tile([128, 2 * CHUNK], F32)
                for half, xnc in ((0, xnc_a), (1, xnc_b)):
                    lo = max(off - half * CHUNK, 0)
                    if lo >= CHUNK:
                        continue
                    gsl = g[:, half * CHUNK + lo:(half + 1) * CHUNK]
                    for kk in range(KP):
                        nc.tensor.matmul(
                            gsl,
                            lhsT=x8_own[:, 2 * kk:2 * kk + 2, 128 * m:128 * (m + 1)],
                            rhs=xnc[:, 2 * kk:2 * kk + 2, lo:CHUNK],
                            start=(kk == 0), stop=(kk == KP - 1), perf_mode=DR,
                        )
                if prev is not None:
                    emit_cs(*prev)
                esb = exp_pool.tile([128, 2 * CHUNK], BF16)
                nc.scalar.activation(
                    esb[:, off:2 * CHUNK], g[:, off:2 * CHUNK], EXP,
                    scale=inv2_rm[:, m:m + 1],
                    accum_out=esum[:, m, t:t + 1],
                )
                if t == 0:
                    dsl = esb[:, 128 * m:128 * (m + 1)]
                    scr = scr_pool.tile([128, 128], F32)
                    nc.vector.tensor_mul(scr[:], dsl, msk_sb[:, 0:128])
                    nc.vector.tensor_reduce(
                        ediag[:, m:m + 1], scr[:],
                        axis=mybir.AxisListType.X, op=ADD)
                    scr2 = scr_pool.tile([128, 128], F32)
                    nc.vector.tensor_mul(scr2[:], dsl, msk_sb[:, 128:256])
                    nc.vector.tensor_reduce(
                        etarg[:, m:m + 1], scr2[:],
                        axis=mybir.AxisListType.X, op=ADD)
                else:
                    prev = (m, esb)
            if t >= 1:
                emit_cs(*prev)
                base = (t - 1) * 2 * CHUNK
                lo_a = 128 if tri else 0
                nc.scalar.copy(csum_sb[0:1, base + lo_a:base + CHUNK],
                               cs_a[0:1, lo_a:CHUNK])
                nc.scalar.copy(csum_sb[0:1, base + CHUNK:base + 2 * CHUNK],
                               cs_b[0:1, :])

        # Pipeline schedule: 2-chunk Newton batches keep each norm pair
        # dependent only on its own two stages (a 4-chunk batch made
        # norm(4,5) wait for stages 6,7 -- a 15 us whole-machine stall);
        # later stages are emitted after earlier sweeps so their squares
        # run under sweep execution on the then-idle DVE/GpSimd.
        raws = {}
        xncs = {}
        for j in range(2):
            raws[j] = stage_chunk(j)
        newton_batch(0, 2)
        for j in range(2, 4):
            raws[j] = stage_chunk(j)
        for j in range(2):
            xncs[j] = norm_chunk(j, raws.pop(j))
        for j in range(4, 6):
            raws[j] = stage_chunk(j)
        newton_batch(2, 2)
        for j in range(2, 4):
            xncs[j] = norm_chunk(j, raws.pop(j))
        sweep(0, xncs.pop(0), xncs.pop(1))
        for j in range(6, 8):
            raws[j] = stage_chunk(j)
        newton_batch(4, 2)
        for j in range(4, 6):
            xncs[j] = norm_chunk(j, raws.pop(j))
        sweep(1, xncs.pop(2), xncs.pop(3))
        for j in range(8, 10):
            raws[j] = stage_chunk(j)
        newton_batch(6, 2)
        for j in range(6, 8):
            xncs[j] = norm_chunk(j, raws.pop(j))
        sweep(2, xncs.pop(4), xncs.pop(5))
        newton_batch(8, 2)
        for j in range(8, 10):
            xncs[j] = norm_chunk(j, raws.pop(j))
        sweep(3, xncs.pop(6), xncs.pop(7))
        sweep(4, xncs.pop(8), xncs.pop(9))

        rsum = small_pool.tile([128, MT], F32)
        nc.vector.tensor_reduce(
            rsum[:], esum[:], axis=mybir.AxisListType.X, op=ADD,
        )
        nc.sync.dma_start(rsum_o[:], rsum[:])
        nc.sync.dma_start(ediag_o[:], ediag[:])
        nc.sync.dma_start(etarg_o[:], etarg[:])
        nc.sync.dma_start(csum_o[:], csum_sb[:])

    nc.finalize()
    return nc


def _get_program():
    if "nc" not in _NC_CACHE:
        _NC_CACHE["nc"] = _build_program()
    return _NC_CACHE["nc"]


def _make_masks():
    m = np.zeros((128, 256), dtype=np.float32)
    p = np.arange(128)
    m[p, p] = 1.0              # identity: diagonal extraction
    m[p, 128 + (p ^ 1)] = 1.0  # pair-swap: target extraction
    return m


def _prep_inputs(z_i, z_j):
    x = np.concatenate([np.asarray(z_i), np.asarray(z_j)], axis=0)
    assert x.shape == (N, D) and x.dtype == np.float32
    xT = np.ascontiguousarray(x.T)  # [D, N]
    x8T = xT.astype(ml_dtypes.float8_e4m3)
    masks = _make_masks()
    in_maps = []
    for c in range(NCORES):
        x8c = np.roll(x8T, -RPC * c, axis=1)[:, :NCH * CHUNK]
        in_maps.append({"x8": np.ascontiguousarray(x8c), "msk": masks})
    return in_maps


def _assemble(results):
    """Host-side final reduction: merge row-sum and column-sum partials,
    then loss = mean(log(den) - log(etarg))."""
    den = np.zeros((NCORES, RPC), dtype=np.float64)
    etarg = np.zeros((NCORES, RPC), dtype=np.float64)
    for c in range(NCORES):
        r = results[c]
        # [128, MT] with row 128m+p at [p, m]
        rs = r["rsum"].astype(np.float64).T.reshape(-1)
        ed = r["ediag"].astype(np.float64).T.reshape(-1)
        et = r["etarg"].astype(np.float64).T.reshape(-1)
        den[c] += rs - ed
        etarg[c] = et
        cs = r["csum"].astype(np.float64).reshape(-1)
        for rblk in range(1, 5):
            part = cs[(rblk - 1) * RPC:(rblk) * RPC]
            den[(c + rblk) % NCORES] += part
    loss_rows = np.log(den) - np.log(etarg)
    return np.float32(loss_rows.mean())


def kernel(z_i: np.ndarray, z_j: np.ndarray, _trace: bool = False) -> np.ndarray:
    global LAST_RESULTS
    nc = _get_program()
    in_maps = _prep_inputs(z_i, z_j)
    res = run_bass_kernel_spmd(
        nc, in_maps, core_ids=list(range(NCORES)), trace=_trace,
    )
    LAST_RESULTS = res
    return _assemble(res.results)


# revision 24
# speedup vs baseline: 1.0239x; 1.0239x over previous
"""NT-Xent loss on 8 Trainium2 NeuronCores (Bass/Tile), fp8 + symmetric.

Reference computation (B=4096, D=1024, T=0.5):
    x  = concat(z_i, z_j); xn = x / ||x||; sim = xn @ xn.T
    logits = sim / T, diag masked to -inf
    loss = -mean(log_softmax(logits)[i, target(i)]), target(i) = i ^ 1

Sharding + symmetry: core c owns rows [1024c, 1024(c+1)). exp(sim/T) is
symmetric, so each core computes only rotated column-blocks r = 0..3
fully plus the upper sub-block triangle of r = 4 (sub-blocks (i,j),
j >= i, of the 8x8 128-col grid). The mirrored contributions are
recovered from per-column sums of the computed exp tiles:
  - blocks r = 1..3: full column sums -> rows of core c+r
  - block r = 4: column sums EXCLUDING the diagonal sub-blocks (those
    pairs are computed by both endpoints' own sweeps) -> rows of c+4
The host adds each core's row-sum partials and the received column-sum
partials, subtracts ediag, and finishes loss = mean(log(den) -
log(etarg)). This is the final cross-core reduction the sharding hint
assigns to an all-reduce; it is O(N) scalar work.

Per-core pipeline: STAGE (DMA fp8 chunk, squares split ACT/DVE/GpSimd,
DoubleRow-ones partition-sum -> s), NEWTON (batched constant-seed
rsqrt on DVE, 4 iterations; s ~ chi^2(1024) so seed 1/32 converges),
NORM (K=1 matmul broadcasts 16/||x||, DVE/GpSimd multiply -> fp8),
SWEEP (8 DoubleRow matmuls per m-tile into a 2-bank [128,1024] PSUM
tile, one ACT exp with per-partition scale inv/8 + f32 row-sum accum;
colsum matmuls lag one m-tile so the in-order PE stream never waits on
ACT). Diag/target extracted from the t=0 pair by mask multiply+reduce.

fp8 rationale: rel-err budget is 2e-2; e4m3 quantization perturbs sim
by ~2e-3 which lands ~2e-5 on the loss. DoubleRow (2 contraction tiles
per pass) is ~1.5x over bf16 at FD=512, and fp8 quarters input DMA.
"""

import numpy as np
import ml_dtypes
from contextlib import ExitStack

import concourse.bass as bass
import concourse.tile as tile
from concourse import bacc, mybir
from concourse.bass_utils import run_bass_kernel_spmd

F32 = mybir.dt.float32
BF16 = mybir.dt.bfloat16
F8 = mybir.dt.float8e4

B = 4096
D = 1024
N = 2 * B            # 8192 rows total
NCORES = 8
RPC = N // NCORES    # 1024 rows per core
KT = D // 128        # 8 contraction partition-tiles
KP = KT // 2         # 4 DoubleRow contraction pairs
MT = RPC // 128      # 8 row tiles per core
CHUNK = 512
NCH = 10             # computed column chunks: blocks r=0..4
NPAIR = 5            # chunk pairs (sweeps)
CSB = 8              # colsum chunks (blocks r=1..4 -> chunks 2..9)

_NC_CACHE = {}
LAST_RESULTS = None  # BassKernelResults of the most recent run (for test.py)


def _build_program():
    nc = bacc.Bacc("TRN2", target_bir_lowering=False, debug=False)

    x8 = nc.dram_tensor("x8", [D, NCH * CHUNK], F8, kind="ExternalInput")
    msk = nc.dram_tensor("msk", [128, 256], F32, kind="ExternalInput")
    rsum_o = nc.dram_tensor("rsum", [128, MT], F32, kind="ExternalOutput")
    ediag_o = nc.dram_tensor("ediag", [128, MT], F32, kind="ExternalOutput")
    etarg_o = nc.dram_tensor("etarg", [128, MT], F32, kind="ExternalOutput")
    csum_o = nc.dram_tensor("csum", [1, CSB * CHUNK], F32, kind="ExternalOutput")

    ADD = mybir.AluOpType.add
    MULT = mybir.AluOpType.mult
    EXP = mybir.ActivationFunctionType.Exp
    SQ = mybir.ActivationFunctionType.Square
    DR = mybir.MatmulPerfMode.DoubleRow

    with tile.TileContext(nc) as tc, ExitStack() as ctx:
        consts = ctx.enter_context(tc.tile_pool(name="consts", bufs=1))
        own_pool = ctx.enter_context(tc.tile_pool(name="own", bufs=1))
        raw_pool = ctx.enter_context(tc.tile_pool(name="raw", bufs=8))
        sq_pool = ctx.enter_context(tc.tile_pool(name="sq", bufs=8))
        xnc_pool = ctx.enter_context(tc.tile_pool(name="xnc", bufs=6))
        sv_pool = ctx.enter_context(tc.tile_pool(name="sv", bufs=4))
        inv_pool = ctx.enter_context(tc.tile_pool(name="invb", bufs=3))
        exp_pool = ctx.enter_context(tc.tile_pool(name="exp", bufs=4))
        scr_pool = ctx.enter_context(tc.tile_pool(name="scr", bufs=2))
        nt_pool = ctx.enter_context(tc.tile_pool(name="nt", bufs=2))
        stat_pool = ctx.enter_context(tc.tile_pool(name="stat", bufs=1))
        dram_pool = ctx.enter_context(tc.tile_pool(name="dram", bufs=1, space="DRAM"))
        small_pool = ctx.enter_context(tc.tile_pool(name="small", bufs=4))
        ps_s = ctx.enter_context(tc.tile_pool(name="ps_s", bufs=1, space="PSUM"))
        ps_b = ctx.enter_context(tc.tile_pool(name="ps_b", bufs=1, space="PSUM"))
        ps_cs = ctx.enter_context(tc.tile_pool(name="ps_cs", bufs=1, space="PSUM"))
        ps_g = ctx.enter_context(tc.tile_pool(name="ps_g", bufs=2, space="PSUM"))

        msk_sb = consts.tile([128, 256], F32)
        nc.sync.dma_start(msk_sb[:], msk[:])
        ones_k1 = consts.tile([1, 128], BF16)
        nc.vector.memset(ones_k1[:], 1.0)
        ones_m1 = consts.tile([128, 1], BF16)
        nc.vector.memset(ones_m1[:], 1.0)
        # DoubleRow ones weights: k-pair step must be 16 B aligned.
        ones_dr = consts.tile([128, 2, 16], F8)
        nc.vector.memset(ones_dr[:], 1.0)

        x8_own = own_pool.tile([128, KT, RPC], F8)

        inv2_rm = stat_pool.tile([128, MT], F32)
        s_dram = dram_pool.tile([1, NCH * CHUNK], F32)
        inv_dram = dram_pool.tile([1, NCH * CHUNK], BF16)

        esum = stat_pool.tile([128, MT, NPAIR], F32)
        ediag = stat_pool.tile([128, MT], F32)
        etarg = stat_pool.tile([128, MT], F32)
        csum_sb = stat_pool.tile([1, CSB * CHUNK], F32)
        # cols [3072, 3200) (head of the r=4 block, no strict-upper source)
        # are never written by the colsum drains; zero everything once.
        nc.vector.memset(csum_sb[:], 0.0)

        x8_r = x8[:].rearrange("(k p) n -> p k n", k=KT)

        def stage_chunk(j):
            """DMA raw fp8 chunk j, square it (DVE/GpSimd/ACT split),
            DoubleRow-ones partition-sum -> s_dram."""
            csl = slice(CHUNK * j, CHUNK * (j + 1))
            if j < 2:
                raw = x8_own[:, :, csl]
            else:
                raw_t = raw_pool.tile([128, KT, CHUNK], F8)
                raw = raw_t[:]
            half = KT // 2
            nc.sync.dma_start(raw[:, 0:half, :], x8_r[:, 0:half, csl])
            nc.sync.dma_start(raw[:, half:KT, :], x8_r[:, half:KT, csl])
            sq = sq_pool.tile([128, KT, CHUNK], F8)
            # engine split; prologue chunks lean on ACT/DVE so the slow
            # GpSimd leg doesn't delay the first Newton batches.
            na, nd = (4, 3) if j < 4 else (3, 2)
            nc.scalar.activation(sq[:, 0:na, :], raw[:, 0:na, :], SQ)
            for k in range(na, na + nd):
                nc.vector.tensor_mul(sq[:, k, :], raw[:, k, :], raw[:, k, :])
            for k in range(na + nd, KT):
                nc.gpsimd.tensor_mul(sq[:, k, :], raw[:, k, :], raw[:, k, :])
            s_ps = ps_s.tile([1, CHUNK], F32)
            for kk in range(KP):
                nc.tensor.matmul(
                    s_ps[:], lhsT=ones_dr[:, :, 0:1], rhs=sq[:, 2 * kk:2 * kk + 2, :],
                    start=(kk == 0), stop=(kk == KP - 1), perf_mode=DR,
                )
            s_sb = sv_pool.tile([1, CHUNK], F32)
            nc.scalar.copy(s_sb[:], s_ps[:])
            nc.gpsimd.dma_start(s_dram[0:1, csl], s_sb[:])
            return raw

        def newton_batch(c0, nch):
            """inv = rsqrt(s) for nch chunks starting at chunk c0, on DVE."""
            base = CHUNK * c0
            bw = nch * CHUNK // 128
            da = s_dram[:]
            s_bat = nt_pool.tile([128, bw], F32)
            nc.gpsimd.dma_start(
                s_bat[:],
                bass.AP(tensor=da.tensor, offset=da.offset + base,
                        ap=[[1, 128], [128, bw]]))
            y = nt_pool.tile([128, bw], F32)
            nc.vector.memset(y[:], 1.0 / 32.0)
            t = nt_pool.tile([128, bw], F32)
            for _ in range(4):
                nc.vector.tensor_mul(t[:], y[:], y[:])
                nc.vector.tensor_mul(t[:], t[:], s_bat[:])
                nc.vector.tensor_scalar(
                    out=t[:], in0=t[:], scalar1=-0.5, scalar2=1.5,
                    op0=MULT, op1=ADD)
                nc.vector.tensor_mul(y[:], y[:], t[:])
            if c0 == 0:
                nc.vector.tensor_scalar_mul(inv2_rm[:], y[:, 0:MT], 0.125)
            y16 = nt_pool.tile([128, bw], BF16)
            nc.vector.tensor_scalar_mul(y16[:], y[:], 16.0)
            di = inv_dram[:]
            nc.gpsimd.dma_start(
                bass.AP(tensor=di.tensor, offset=di.offset + base,
                        ap=[[1, 128], [128, bw]]),
                y16[:])

        def norm_chunk(j, raw):
            csl = slice(CHUNK * j, CHUNK * (j + 1))
            inv_sl = sv_pool.tile([1, CHUNK], BF16)
            nc.sync.dma_start(inv_sl[:], inv_dram[0:1, csl])
            b_ps = ps_b.tile([128, CHUNK], F32)
            nc.tensor.matmul(b_ps[:], lhsT=ones_k1[:], rhs=inv_sl[:],
                             start=True, stop=True)
            invn = inv_pool.tile([128, CHUNK], BF16)
            nc.scalar.copy(invn[:], b_ps[:])
            xnc = xnc_pool.tile([128, KT, CHUNK], F8)
            nd = 4 if j % 2 == 0 else 5
            for k in range(nd):
                nc.vector.tensor_mul(xnc[:, k, :], raw[:, k, :], invn[:])
            for k in range(nd, KT):
                nc.gpsimd.tensor_mul(xnc[:, k, :], raw[:, k, :], invn[:])
            return xnc

        def sweep(t, xnc_a, xnc_b):
            """m-tiles against chunk pair (2t, 2t+1). t=4 is the block-4
            triangle: m-tile m covers block-local cols [128m, 1024).
            Colsums (pairs t>=1) accumulate over m in PSUM; t=4 colsums
            exclude the diagonal sub-block of each m. The colsum matmul
            for m is emitted after sim(m+1) so the in-order PE stream
            never waits on ACT's exp(m)."""
            tri = (t == NPAIR - 1)
            if t >= 1:
                cs_a = ps_cs.tile([1, CHUNK], F32)
                cs_b = ps_cs.tile([1, CHUNK], F32)

            def emit_cs(m, esb):
                # column sums for the mirrored rows. For the triangle
                # pair, skip the diagonal sub-block: start at 128(m+1).
                cs_off = 128 * (m + 1) if tri else 0
                for half, cs in ((0, cs_a), (1, cs_b)):
                    lo = max(cs_off - half * CHUNK, 0)
                    if lo >= CHUNK:
                        continue
                    first_m = 0
                    last_m = (2 if half == 0 else 6) if tri else MT - 1
                    if m > last_m:
                        continue
                    nc.tensor.matmul(
                        cs[0:1, lo:CHUNK], lhsT=ones_m1[:],
                        rhs=esb[:, half * CHUNK + lo:(half + 1) * CHUNK],
                        start=(m == first_m), stop=(m == last_m),
                        skip_group_check=True,
                    )

            prev = None
            for m in range(MT):
                off = 128 * m if tri else 0   # block-local start col
                g = ps_g.tile([128, 2 * CHUNK], F32)
                for half, xnc in ((0, xnc_a), (1, xnc_b)):
                    lo = max(off - half * CHUNK, 0)
                    if lo >= CHUNK:
                        continue
                    gsl = g[:, half * CHUNK + lo:(half + 1) * CHUNK]
                    for kk in range(KP):
                        nc.tensor.matmul(
                            gsl,
                            lhsT=x8_own[:, 2 * kk:2 * kk + 2, 128 * m:128 * (m + 1)],
                            rhs=xnc[:, 2 * kk:2 * kk + 2, lo:CHUNK],
                            start=(kk == 0), stop=(kk == KP - 1), perf_mode=DR,
                        )
                if prev is not None:
                    emit_cs(*prev)
                esb = exp_pool.tile([128, 2 * CHUNK], BF16)
                nc.scalar.activation(
                    esb[:, off:2 * CHUNK], g[:, off:2 * CHUNK], EXP,
                    scale=inv2_rm[:, m:m + 1],
                    accum_out=esum[:, m, t:t + 1],
                )
                if t == 0:
                    dsl = esb[:, 128 * m:128 * (m + 1)]
                    scr = scr_pool.tile([128, 128], F32)
                    nc.vector.tensor_mul(scr[:], dsl, msk_sb[:, 0:128])
                    nc.vector.tensor_reduce(
                        ediag[:, m:m + 1], scr[:],
                        axis=mybir.AxisListType.X, op=ADD)
                    scr2 = scr_pool.tile([128, 128], F32)
                    nc.vector.tensor_mul(scr2[:], dsl, msk_sb[:, 128:256])
                    nc.vector.tensor_reduce(
                        etarg[:, m:m + 1], scr2[:],
                        axis=mybir.AxisListType.X, op=ADD)
                else:
                    prev = (m, esb)
            if t >= 1:
                emit_cs(*prev)
                base = (t - 1) * 2 * CHUNK
                lo_a = 128 if tri else 0
                nc.scalar.copy(csum_sb[0:1, base + lo_a:base + CHUNK],
                               cs_a[0:1, lo_a:CHUNK])
                nc.scalar.copy(csum_sb[0:1, base + CHUNK:base + 2 * CHUNK],
                               cs_b[0:1, :])

        # Pipeline schedule: early 2-chunk Newton batches shorten the
        # prologue before sweep(0); stages run well ahead of the sweeps
        # that consume them so the slower elementwise engines keep a
        # head start on the PE. (Emitting stages later, under the
        # sweeps, measured WORSE: the DVE/GpSimd backlog then stalls the
        # in-order PE at the inter-sweep s/broadcast matmuls.)
        raws = {}
        xncs = {}
        for j in range(2):
            raws[j] = stage_chunk(j)
        newton_batch(0, 2)
        for j in range(2, 4):
            raws[j] = stage_chunk(j)
        for j in range(2):
            xncs[j] = norm_chunk(j, raws.pop(j))
        for j in range(4, 6):
            raws[j] = stage_chunk(j)
        newton_batch(2, 2)
        for j in range(2, 4):
            xncs[j] = norm_chunk(j, raws.pop(j))
        sweep(0, xncs.pop(0), xncs.pop(1))
        for j in range(6, 8):
            raws[j] = stage_chunk(j)
        newton_batch(4, 4)
        for j in range(4, 6):
            xncs[j] = norm_chunk(j, raws.pop(j))
        sweep(1, xncs.pop(2), xncs.pop(3))
        for j in range(8, 10):
            raws[j] = stage_chunk(j)
        for j in range(6, 8):
            xncs[j] = norm_chunk(j, raws.pop(j))
        sweep(2, xncs.pop(4), xncs.pop(5))
        newton_batch(8, 2)
        for j in range(8, 10):
            xncs[j] = norm_chunk(j, raws.pop(j))
        sweep(3, xncs.pop(6), xncs.pop(7))
        sweep(4, xncs.pop(8), xncs.pop(9))

        rsum = small_pool.tile([128, MT], F32)
        nc.vector.tensor_reduce(
            rsum[:], esum[:], axis=mybir.AxisListType.X, op=ADD,
        )
        nc.sync.dma_start(rsum_o[:], rsum[:])
        nc.sync.dma_start(ediag_o[:], ediag[:])
        nc.sync.dma_start(etarg_o[:], etarg[:])
        nc.sync.dma_start(csum_o[:], csum_sb[:])

    nc.finalize()
    return nc


def _get_program():
    if "nc" not in _NC_CACHE:
        _NC_CACHE["nc"] = _build_program()
    return _NC_CACHE["nc"]


def _make_masks():
    m = np.zeros((128, 256), dtype=np.float32)
    p = np.arange(128)
    m[p, p] = 1.0              # identity: diagonal extraction
    m[p, 128 + (p ^ 1)] = 1.0  # pair-swap: target extraction
    return m


def _prep_inputs(z_i, z_j):
    x = np.concatenate([np.asarray(z_i), np.asarray(z_j)], axis=0)
    assert x.shape == (N, D) and x.dtype == np.float32
    xT = np.ascontiguousarray(x.T)  # [D, N]
    x8T = xT.astype(ml_dtypes.float8_e4m3)
    masks = _make_masks()
    in_maps = []
    for c in range(NCORES):
        x8c = np.roll(x8T, -RPC * c, axis=1)[:, :NCH * CHUNK]
        in_maps.append({"x8": np.ascontiguousarray(x8c), "msk": masks})
    return in_maps


def _assemble(results):
    """Host-side final reduction: merge row-sum and column-sum partials,
    then loss = mean(log(den) - log(etarg))."""
    den = np.zeros((NCORES, RPC), dtype=np.float64)
    etarg = np.zeros((NCORES, RPC), dtype=np.float64)
    for c in range(NCORES):
        r = results[c]
        # [128, MT] with row 128m+p at [p, m]
        rs = r["rsum"].astype(np.float64).T.reshape(-1)
        ed = r["ediag"].astype(np.float64).T.reshape(-1)
        et = r["etarg"].astype(np.float64).T.reshape(-1)
        den[c] += rs - ed
        etarg[c] = et
        cs = r["csum"].astype(np.float64).reshape(-1)
        for rblk in range(1, 5):
            part = cs[(rblk - 1) * RPC:(rblk) * RPC]
            den[(c + rblk) % NCORES] += part
    loss_rows = np.log(den) - np.log(etarg)
    return np.float32(loss_rows.mean())


def kernel(z_i: np.ndarray, z_j: np.ndarray, _trace: bool = False) -> np.ndarray:
    global LAST_RESULTS
    nc = _get_program()
    in_maps = _prep_inputs(z_i, z_j)
    res = run_bass_kernel_spmd(
        nc, in_maps, core_ids=list(range(NCORES)), trace=_trace,
    )
    LAST_RESULTS = res
    return _assemble(res.results)


# revision 26
# speedup vs baseline: 1.0452x; 1.0208x over previous
"""NT-Xent loss on 8 Trainium2 NeuronCores (Bass/Tile), fp8 + symmetric.

Reference computation (B=4096, D=1024, T=0.5):
    x  = concat(z_i, z_j); xn = x / ||x||; sim = xn @ xn.T
    logits = sim / T, diag masked to -inf
    loss = -mean(log_softmax(logits)[i, target(i)]), target(i) = i ^ 1

Sharding + symmetry: core c owns rows [1024c, 1024(c+1)). exp(sim/T) is
symmetric, so each core computes only rotated column-blocks r = 0..3
fully plus the upper sub-block triangle of r = 4 (sub-blocks (i,j),
j >= i, of the 8x8 128-col grid). The mirrored contributions are
recovered from per-column sums of the computed exp tiles:
  - blocks r = 1..3: full column sums -> rows of core c+r
  - block r = 4: column sums EXCLUDING the diagonal sub-blocks (those
    pairs are computed by both endpoints' own sweeps) -> rows of c+4
The host adds each core's row-sum partials and the received column-sum
partials, subtracts ediag, and finishes loss = mean(log(den) -
log(etarg)). This is the final cross-core reduction the sharding hint
assigns to an all-reduce; it is O(N) scalar work.

Per-core pipeline: STAGE (DMA fp8 chunk, squares split ACT/DVE/GpSimd,
DoubleRow-ones partition-sum -> s), NEWTON (batched constant-seed
rsqrt on DVE, 4 iterations; s ~ chi^2(1024) so seed 1/32 converges),
NORM (K=1 matmul broadcasts 16/||x||, DVE/GpSimd multiply -> fp8),
SWEEP (8 DoubleRow matmuls per m-tile into a 2-bank [128,1024] PSUM
tile, one ACT exp with per-partition scale inv/8 + f32 row-sum accum;
colsum matmuls lag one m-tile so the in-order PE stream never waits on
ACT). Diag/target extracted from the t=0 pair by mask multiply+reduce.

fp8 rationale: rel-err budget is 2e-2; e4m3 quantization perturbs sim
by ~2e-3 which lands ~2e-5 on the loss. DoubleRow (2 contraction tiles
per pass) is ~1.5x over bf16 at FD=512, and fp8 quarters input DMA.
"""

import numpy as np
import ml_dtypes
from contextlib import ExitStack

import concourse.bass as bass
import concourse.tile as tile
from concourse import bacc, mybir
from concourse.bass_utils import run_bass_kernel_spmd

F32 = mybir.dt.float32
BF16 = mybir.dt.bfloat16
F8 = mybir.dt.float8e4

B = 4096
D = 1024
N = 2 * B            # 8192 rows total
NCORES = 8
RPC = N // NCORES    # 1024 rows per core
KT = D // 128        # 8 contraction partition-tiles
KP = KT // 2         # 4 DoubleRow contraction pairs
MT = RPC // 128      # 8 row tiles per core
CHUNK = 512
NCH = 10             # computed column chunks: blocks r=0..4
NPAIR = 5            # chunk pairs (sweeps)
CSB = 8              # colsum chunks (blocks r=1..4 -> chunks 2..9)

_NC_CACHE = {}
LAST_RESULTS = None  # BassKernelResults of the most recent run (for test.py)


def _build_program():
    nc = bacc.Bacc("TRN2", target_bir_lowering=False, debug=False)

    x8 = nc.dram_tensor("x8", [D, NCH * CHUNK], F8, kind="ExternalInput")
    msk = nc.dram_tensor("msk", [128, 256], F32, kind="ExternalInput")
    rsum_o = nc.dram_tensor("rsum", [128, MT], F32, kind="ExternalOutput")
    ediag_o = nc.dram_tensor("ediag", [128, MT], F32, kind="ExternalOutput")
    etarg_o = nc.dram_tensor("etarg", [128, MT], F32, kind="ExternalOutput")
    csum_o = nc.dram_tensor("csum", [1, CSB * CHUNK], F32, kind="ExternalOutput")

    ADD = mybir.AluOpType.add
    MULT = mybir.AluOpType.mult
    EXP = mybir.ActivationFunctionType.Exp
    SQ = mybir.ActivationFunctionType.Square
    DR = mybir.MatmulPerfMode.DoubleRow

    with tile.TileContext(nc) as tc, ExitStack() as ctx:
        consts = ctx.enter_context(tc.tile_pool(name="consts", bufs=1))
        own_pool = ctx.enter_context(tc.tile_pool(name="own", bufs=1))
        raw_pool = ctx.enter_context(tc.tile_pool(name="raw", bufs=8))
        sq_pool = ctx.enter_context(tc.tile_pool(name="sq", bufs=8))
        xnc_pool = ctx.enter_context(tc.tile_pool(name="xnc", bufs=6))
        sv_pool = ctx.enter_context(tc.tile_pool(name="sv", bufs=4))
        inv_pool = ctx.enter_context(tc.tile_pool(name="invb", bufs=3))
        exp_pool = ctx.enter_context(tc.tile_pool(name="exp", bufs=4))
        scr_pool = ctx.enter_context(tc.tile_pool(name="scr", bufs=2))
        nt_pool = ctx.enter_context(tc.tile_pool(name="nt", bufs=2))
        stat_pool = ctx.enter_context(tc.tile_pool(name="stat", bufs=1))
        dram_pool = ctx.enter_context(tc.tile_pool(name="dram", bufs=1, space="DRAM"))
        small_pool = ctx.enter_context(tc.tile_pool(name="small", bufs=4))
        ps_s = ctx.enter_context(tc.tile_pool(name="ps_s", bufs=1, space="PSUM"))
        ps_b = ctx.enter_context(tc.tile_pool(name="ps_b", bufs=1, space="PSUM"))
        ps_cs = ctx.enter_context(tc.tile_pool(name="ps_cs", bufs=1, space="PSUM"))
        ps_g = ctx.enter_context(tc.tile_pool(name="ps_g", bufs=2, space="PSUM"))

        msk_sb = consts.tile([128, 256], F32)
        nc.sync.dma_start(msk_sb[:], msk[:])
        ones_k1 = consts.tile([1, 128], BF16)
        nc.vector.memset(ones_k1[:], 1.0)
        ones_m1 = consts.tile([128, 1], BF16)
        nc.vector.memset(ones_m1[:], 1.0)
        # DoubleRow ones weights: k-pair step must be 16 B aligned.
        ones_dr = consts.tile([128, 2, 16], F8)
        nc.vector.memset(ones_dr[:], 1.0)

        x8_own = own_pool.tile([128, KT, RPC], F8)

        inv2_rm = stat_pool.tile([128, MT], F32)
        s_dram = dram_pool.tile([1, NCH * CHUNK], F32)
        inv_dram = dram_pool.tile([1, NCH * CHUNK], BF16)

        esum = stat_pool.tile([128, MT, NPAIR], F32)
        ediag = stat_pool.tile([128, MT], F32)
        etarg = stat_pool.tile([128, MT], F32)
        csum_sb = stat_pool.tile([1, CSB * CHUNK], F32)
        # cols [3072, 3200) (head of the r=4 block, no strict-upper source)
        # are never written by the colsum drains; zero everything once.
        nc.vector.memset(csum_sb[:], 0.0)

        x8_r = x8[:].rearrange("(k p) n -> p k n", k=KT)

        def stage_chunk(j):
            """DMA raw fp8 chunk j, square it (DVE/GpSimd/ACT split),
            DoubleRow-ones partition-sum -> s_dram."""
            csl = slice(CHUNK * j, CHUNK * (j + 1))
            if j < 2:
                raw = x8_own[:, :, csl]
            else:
                raw_t = raw_pool.tile([128, KT, CHUNK], F8)
                raw = raw_t[:]
            half = KT // 2
            nc.sync.dma_start(raw[:, 0:half, :], x8_r[:, 0:half, csl])
            nc.sync.dma_start(raw[:, half:KT, :], x8_r[:, half:KT, csl])
            sq = sq_pool.tile([128, KT, CHUNK], F8)
            # engine split; prologue chunks lean on ACT/DVE so the slow
            # GpSimd leg doesn't delay the first Newton batches.
            na, nd = (4, 3) if j < 4 else (4, 2)
            nc.scalar.activation(sq[:, 0:na, :], raw[:, 0:na, :], SQ)
            for k in range(na, na + nd):
                nc.vector.tensor_mul(sq[:, k, :], raw[:, k, :], raw[:, k, :])
            for k in range(na + nd, KT):
                nc.gpsimd.tensor_mul(sq[:, k, :], raw[:, k, :], raw[:, k, :])
            s_ps = ps_s.tile([1, CHUNK], F32)
            for kk in range(KP):
                nc.tensor.matmul(
                    s_ps[:], lhsT=ones_dr[:, :, 0:1], rhs=sq[:, 2 * kk:2 * kk + 2, :],
                    start=(kk == 0), stop=(kk == KP - 1), perf_mode=DR,
                )
            s_sb = sv_pool.tile([1, CHUNK], F32)
            nc.scalar.copy(s_sb[:], s_ps[:])
            nc.gpsimd.dma_start(s_dram[0:1, csl], s_sb[:])
            return raw

        def newton_batch(c0, nch):
            """inv = rsqrt(s) for nch chunks starting at chunk c0, on DVE."""
            base = CHUNK * c0
            bw = nch * CHUNK // 128
            da = s_dram[:]
            s_bat = nt_pool.tile([128, bw], F32)
            nc.gpsimd.dma_start(
                s_bat[:],
                bass.AP(tensor=da.tensor, offset=da.offset + base,
                        ap=[[1, 128], [128, bw]]))
            y = nt_pool.tile([128, bw], F32)
            nc.vector.memset(y[:], 1.0 / 32.0)
            t = nt_pool.tile([128, bw], F32)
            for _ in range(4):
                nc.vector.tensor_mul(t[:], y[:], y[:])
                nc.vector.tensor_mul(t[:], t[:], s_bat[:])
                nc.vector.tensor_scalar(
                    out=t[:], in0=t[:], scalar1=-0.5, scalar2=1.5,
                    op0=MULT, op1=ADD)
                nc.vector.tensor_mul(y[:], y[:], t[:])
            if c0 == 0:
                nc.vector.tensor_scalar_mul(inv2_rm[:], y[:, 0:MT], 0.125)
            y16 = nt_pool.tile([128, bw], BF16)
            nc.vector.tensor_scalar_mul(y16[:], y[:], 16.0)
            di = inv_dram[:]
            nc.gpsimd.dma_start(
                bass.AP(tensor=di.tensor, offset=di.offset + base,
                        ap=[[1, 128], [128, bw]]),
                y16[:])

        def norm_chunk(j, raw):
            csl = slice(CHUNK * j, CHUNK * (j + 1))
            inv_sl = sv_pool.tile([1, CHUNK], BF16)
            nc.sync.dma_start(inv_sl[:], inv_dram[0:1, csl])
            b_ps = ps_b.tile([128, CHUNK], F32)
            nc.tensor.matmul(b_ps[:], lhsT=ones_k1[:], rhs=inv_sl[:],
                             start=True, stop=True)
            invn = inv_pool.tile([128, CHUNK], BF16)
            nc.scalar.copy(invn[:], b_ps[:])
            xnc = xnc_pool.tile([128, KT, CHUNK], F8)
            nd = 4 if j % 2 == 0 else 5
            for k in range(nd):
                nc.vector.tensor_mul(xnc[:, k, :], raw[:, k, :], invn[:])
            for k in range(nd, KT):
                nc.gpsimd.tensor_mul(xnc[:, k, :], raw[:, k, :], invn[:])
            return xnc

        def sweep(t, xnc_a, xnc_b):
            """m-tiles against chunk pair (2t, 2t+1). t=4 is the block-4
            triangle: m-tile m covers block-local cols [128m, 1024).
            Colsums (pairs t>=1) accumulate over m in PSUM; t=4 colsums
            exclude the diagonal sub-block of each m. The colsum matmul
            for m is emitted after sim(m+1) so the in-order PE stream
            never waits on ACT's exp(m)."""
            tri = (t == NPAIR - 1)
            if t >= 1:
                cs_a = ps_cs.tile([1, CHUNK], F32)
                cs_b = ps_cs.tile([1, CHUNK], F32)

            def emit_cs(m, esb):
                # column sums for the mirrored rows. For the triangle
                # pair, skip the diagonal sub-block: start at 128(m+1).
                cs_off = 128 * (m + 1) if tri else 0
                for half, cs in ((0, cs_a), (1, cs_b)):
                    lo = max(cs_off - half * CHUNK, 0)
                    if lo >= CHUNK:
                        continue
                    first_m = 0
                    last_m = (2 if half == 0 else 6) if tri else MT - 1
                    if m > last_m:
                        continue
                    nc.tensor.matmul(
                        cs[0:1, lo:CHUNK], lhsT=ones_m1[:],
                        rhs=esb[:, half * CHUNK + lo:(half + 1) * CHUNK],
                        start=(m == first_m), stop=(m == last_m),
                        skip_group_check=True,
                    )

            prev = None
            for m in range(MT):
                off = 128 * m if tri else 0   # block-local start col
                g = ps_g.tile([128, 2 * CHUNK], F32)
                for half, xnc in ((0, xnc_a), (1, xnc_b)):
                    lo = max(off - half * CHUNK, 0)
                    if lo >= CHUNK:
                        continue
                    gsl = g[:, half * CHUNK + lo:(half + 1) * CHUNK]
                    for kk in range(KP):
                        nc.tensor.matmul(
                            gsl,
                            lhsT=x8_own[:, 2 * kk:2 * kk + 2, 128 * m:128 * (m + 1)],
                            rhs=xnc[:, 2 * kk:2 * kk + 2, lo:CHUNK],
                            start=(kk == 0), stop=(kk == KP - 1), perf_mode=DR,
                        )
                if prev is not None:
                    emit_cs(*prev)
                esb = exp_pool.tile([128, 2 * CHUNK], BF16)
                nc.scalar.activation(
                    esb[:, off:2 * CHUNK], g[:, off:2 * CHUNK], EXP,
                    scale=inv2_rm[:, m:m + 1],
                    accum_out=esum[:, m, t:t + 1],
                )
                if t == 0:
                    dsl = esb[:, 128 * m:128 * (m + 1)]
                    scr = scr_pool.tile([128, 128], F32)
                    nc.vector.tensor_mul(scr[:], dsl, msk_sb[:, 0:128])
                    nc.vector.tensor_reduce(
                        ediag[:, m:m + 1], scr[:],
                        axis=mybir.AxisListType.X, op=ADD)
                    scr2 = scr_pool.tile([128, 128], F32)
                    nc.vector.tensor_mul(scr2[:], dsl, msk_sb[:, 128:256])
                    nc.vector.tensor_reduce(
                        etarg[:, m:m + 1], scr2[:],
                        axis=mybir.AxisListType.X, op=ADD)
                else:
                    prev = (m, esb)
            if t >= 1:
                emit_cs(*prev)
                base = (t - 1) * 2 * CHUNK
                lo_a = 128 if tri else 0
                nc.scalar.copy(csum_sb[0:1, base + lo_a:base + CHUNK],
                               cs_a[0:1, lo_a:CHUNK])
                nc.scalar.copy(csum_sb[0:1, base + CHUNK:base + 2 * CHUNK],
                               cs_b[0:1, :])

        # Pipeline schedule: early 2-chunk Newton batches shorten the
        # prologue before sweep(0); stages run well ahead of the sweeps
        # that consume them so the slower elementwise engines keep a
        # head start on the PE. (Emitting stages later, under the
        # sweeps, measured WORSE: the DVE/GpSimd backlog then stalls the
        # in-order PE at the inter-sweep s/broadcast matmuls.)
        raws = {}
        xncs = {}
        for j in range(2):
            raws[j] = stage_chunk(j)
        newton_batch(0, 2)
        for j in range(2, 4):
            raws[j] = stage_chunk(j)
        for j in range(2):
            xncs[j] = norm_chunk(j, raws.pop(j))
        for j in range(4, 6):
            raws[j] = stage_chunk(j)
        newton_batch(2, 2)
        for j in range(2, 4):
            xncs[j] = norm_chunk(j, raws.pop(j))
        sweep(0, xncs.pop(0), xncs.pop(1))
        nc.sync.dma_start(ediag_o[:], ediag[:])
        nc.sync.dma_start(etarg_o[:], etarg[:])
        newton_batch(4, 2)
        for j in range(6, 8):
            raws[j] = stage_chunk(j)
        for j in range(4, 6):
            xncs[j] = norm_chunk(j, raws.pop(j))
        sweep(1, xncs.pop(2), xncs.pop(3))
        for j in range(8, 10):
            raws[j] = stage_chunk(j)
        newton_batch(6, 2)
        for j in range(6, 8):
            xncs[j] = norm_chunk(j, raws.pop(j))
        sweep(2, xncs.pop(4), xncs.pop(5))
        newton_batch(8, 2)
        for j in range(8, 10):
            xncs[j] = norm_chunk(j, raws.pop(j))
        sweep(3, xncs.pop(6), xncs.pop(7))
        sweep(4, xncs.pop(8), xncs.pop(9))

        rsum = small_pool.tile([128, MT], F32)
        nc.vector.tensor_reduce(
            rsum[:], esum[:], axis=mybir.AxisListType.X, op=ADD,
        )
        nc.sync.dma_start(rsum_o[:], rsum[:])
        nc.sync.dma_start(csum_o[:], csum_sb[:])

    nc.finalize()
    return nc


def _get_program():
    if "nc" not in _NC_CACHE:
        _NC_CACHE["nc"] = _build_program()
    return _NC_CACHE["nc"]


def _make_masks():
    m = np.zeros((128, 256), dtype=np.float32)
    p = np.arange(128)
    m[p, p] = 1.0              # identity: diagonal extraction
    m[p, 128 + (p ^ 1)] = 1.0  # pair-swap: target extraction
    return m


def _prep_inputs(z_i, z_j):
    x = np.concatenate([np.asarray(z_i), np.asarray(z_j)], axis=0)
    assert x.shape == (N, D) and x.dtype == np.float32
    xT = np.ascontiguousarray(x.T)  # [D, N]
    x8T = xT.astype(ml_dtypes.float8_e4m3)
    masks = _make_masks()
    in_maps = []
    for c in range(NCORES):
        x8c = np.roll(x8T, -RPC * c, axis=1)[:, :NCH * CHUNK]
        in_maps.append({"x8": np.ascontiguousarray(x8c), "msk": masks})
    return in_maps


def _assemble(results):
    """Host-side final reduction: merge row-sum and column-sum partials,
    then loss = mean(log(den) - log(etarg))."""
    den = np.zeros((NCORES, RPC), dtype=np.float64)
    etarg = np.zeros((NCORES, RPC), dtype=np.float64)
    for c in range(NCORES):
        r = results[c]
        # [128, MT] with row 128m+p at [p, m]
        rs = r["rsum"].astype(np.float64).T.reshape(-1)
        ed = r["ediag"].astype(np.float64).T.reshape(-1)
        et = r["etarg"].astype(np.float64).T.reshape(-1)
        den[c] += rs - ed
        etarg[c] = et
        cs = r["csum"].astype(np.float64).reshape(-1)
        for rblk in range(1, 5):
            part = cs[(rblk - 1) * RPC:(rblk) * RPC]
            den[(c + rblk) % NCORES] += part
    loss_rows = np.log(den) - np.log(etarg)
    return np.float32(loss_rows.mean())


def kernel(z_i: np.ndarray, z_j: np.ndarray, _trace: bool = False) -> np.ndarray:
    global LAST_RESULTS
    nc = _get_program()
    in_maps = _prep_inputs(z_i, z_j)
    res = run_bass_kernel_spmd(
        nc, in_maps, core_ids=list(range(NCORES)), trace=_trace,
    )
    LAST_RESULTS = res
    return _assemble(res.results)
